# revision 1
# baseline (speedup 1.0000x reference)
"""Block-diagonal rotation (COB) kernel for Trainium2, 8 NeuronCores.

Computes out[..., block_i] = x[..., block_i] @ W_i.T for 8 square blocks of
sizes [512, 1024, 256, 768, 384, 640, 128, 384] (features sum to 4096),
x shape (4, 2048, 4096) fp32.

Strategy:
  - Pure data-parallel over rows: 8192 rows are split 8 ways (1024 rows/core).
    Each core gets all (host-pre-transposed) weights.
  - Weights are DMA'd once into SBUF and stay resident as float32r
    (TRN2's fast 4-byte matmul dtype: 1 cycle/row vs 4 for fp32,
    ~1.5e-4 max rel err at these contraction depths).
  - x tiles [128, 4096] are DMA'd naturally (rows on partitions), transposed
    128x128 on the TensorEngine (transpose mode), PSUM->SBUF copied by the
    VectorEngine, then used as the stationary operand of f32r matmuls
    against the resident weight tiles.  PSUM accumulates over each block's
    contraction dim; results are copied to an SBUF staging tile and DMA'd
    out in one 2 MiB transfer per 128-row tile.
  - fp32 bits are fed directly into float32r tiles (verified bit-identical
    to explicitly rounded operands on HW - the PE rounds internally).
"""

import numpy as np

import concourse.bacc as bacc
import concourse.mybir as mybir
from concourse.tile import TileContext
from concourse.bass_utils import run_bass_kernel_spmd
from concourse.masks import make_identity

SIZES = [512, 1024, 256, 768, 384, 640, 128, 384]
OFFS = np.cumsum([0] + SIZES)
N_CORES = 8
ROWS_TOTAL = 4 * 2048
ROWS_PER_CORE = ROWS_TOTAL // N_CORES  # 1024
D = 4096
P = 128
R_TILES = ROWS_PER_CORE // P  # 8

# e-slices per block: chunks <=512, all >=256 when possible (f32r matmul
# runs 1 cycle/row only for moving dim >= 256; 512 is the PSUM bank limit)
E_SLICES = {
    512: [512], 1024: [512, 512], 256: [256], 768: [512, 256],
    384: [384], 640: [384, 256], 128: [128],
}

F32R = mybir.dt.float32r
F32 = mybir.dt.float32

_cache = {}


def build_nc():
    if "nc" in _cache:
        return _cache["nc"]
    nc = bacc.Bacc()
    x_d = nc.declare_dram_parameter("x", [ROWS_PER_CORE, D], F32R, isOutput=False)
    w_d = [
        nc.declare_dram_parameter(f"w{i}", [s, s], F32R, isOutput=False)
        for i, s in enumerate(SIZES)
    ]
    out_d = nc.declare_dram_parameter("out", [ROWS_PER_CORE, D], F32, isOutput=True)

    x_v = x_d.rearrange("(r p) d -> r p d", p=P)
    out_v = out_d.rearrange("(r p) d -> r p d", p=P)

    with TileContext(nc) as tc:
        with (
            tc.tile_pool(name="wres", bufs=1) as wres,
            tc.tile_pool(name="xnat", bufs=2) as xnat_p,
            tc.tile_pool(name="xt", bufs=2) as xt_p,
            tc.tile_pool(name="osb", bufs=2) as osb_p,
            tc.tile_pool(name="idp", bufs=1) as idp,
            tc.tile_pool(name="tp", bufs=2, space="PSUM") as tp_p,
            tc.tile_pool(name="mm", bufs=4, space="PSUM") as mm_p,
        ):
            # identity (f32r) for PE transpose
            id32 = idp.tile([P, P], F32, tag="id32")
            make_identity(nc, id32[:])
            ident = idp.tile([P, P], F32R, tag="idr")
            nc.vector.tensor_copy(ident[:], id32[:])

            # resident weights: per block, per k-tile: [128, s] f32r
            wt = []
            for i, s in enumerate(SIZES):
                w_v = w_d[i].rearrange("(k p) e -> k p e", p=P)
                ks = []
                for k in range(s // P):
                    t = wres.tile([P, s], F32R, tag=f"w{i}_{k}")
                    nc.sync.dma_start(out=t[:], in_=w_v[k])
                    ks.append(t)
                wt.append(ks)

            for r in range(R_TILES):
                x_t = xnat_p.tile([P, D], F32R, tag="xn")
                nc.sync.dma_start(out=x_t[:], in_=x_v[r])

                # transpose all 32 d-tiles: xts[j][:, 128*i:...] holds
                # d-tile (4j+i) as [d on partitions, r free]
                xts = []
                for j in range(D // (4 * P)):  # 8 groups of 4
                    ps = tp_p.tile([P, 4 * P], F32R, tag="tp")
                    for i in range(4):
                        nc.tensor.transpose(
                            ps[:, P * i:P * (i + 1)],
                            x_t[:, P * (4 * j + i):P * (4 * j + i + 1)],
                            ident[:],
                        )
                    xt = xt_p.tile([P, 4 * P], F32R, tag=f"xt{j}")
                    nc.vector.tensor_copy(xt[:], ps[:])
                    xts.append(xt)

                o_t = osb_p.tile([P, D], F32, tag="os")
                for b, s in enumerate(SIZES):
                    d0 = int(OFFS[b]) // P  # first global d-tile of block
                    kt = s // P
                    n0 = 0
                    for nw in E_SLICES[s]:
                        ps = mm_p.tile([P, nw], F32, tag="mm")
                        for k in range(kt):
                            g = d0 + k
                            lhsT = xts[g // 4][:, P * (g % 4):P * (g % 4 + 1)]
                            nc.tensor.matmul(
                                ps[:], lhsT, wt[b][k][:, n0:n0 + nw],
                                start=(k == 0), stop=(k == kt - 1),
                            )
                        nc.vector.tensor_copy(
                            o_t[:, int(OFFS[b]) + n0:int(OFFS[b]) + n0 + nw], ps[:]
                        )
                        n0 += nw
                nc.sync.dma_start(out=out_v[r], in_=o_t[:])

    nc.finalize()
    _cache["nc"] = nc
    return nc


def build_in_maps(x, w0, w1, w2, w3, w4, w5, w6, w7):
    x = np.ascontiguousarray(np.asarray(x, dtype=np.float32)).reshape(ROWS_TOTAL, D)
    ws = [w0, w1, w2, w3, w4, w5, w6, w7]
    wts = [
        np.ascontiguousarray(np.asarray(w, dtype=np.float32).T) for w in ws
    ]
    in_maps = []
    for c in range(N_CORES):
        m = {"x": x[c * ROWS_PER_CORE:(c + 1) * ROWS_PER_CORE]}
        for i, wt in enumerate(wts):
            m[f"w{i}"] = wt
        in_maps.append(m)
    return in_maps


def kernel(x, w0, w1, w2, w3, w4, w5, w6, w7):
    nc = build_nc()
    in_maps = build_in_maps(x, w0, w1, w2, w3, w4, w5, w6, w7)
    res = run_bass_kernel_spmd(nc, in_maps, list(range(N_CORES)))
    out = np.concatenate([r["out"] for r in res.results], axis=0)
    return out.reshape(4, 2048, D).astype(np.float32, copy=False)


# revision 3
# speedup vs baseline: 1.0467x; 1.0467x over previous
"""Block-diagonal rotation (COB) kernel for Trainium2, 8 NeuronCores.

Computes out[..., block_i] = x[..., block_i] @ W_i.T for 8 square blocks of
sizes [512, 1024, 256, 768, 384, 640, 128, 384] (features sum to 4096),
x shape (4, 2048, 4096) fp32.

Strategy:
  - Pure data-parallel over rows: 8192 rows are split 8 ways (1024 rows/core).
    Each core gets all (host-pre-transposed) weights.
  - Weights are DMA'd once into SBUF and stay resident as float32r
    (TRN2's fast 4-byte matmul dtype: 1 cycle/row vs 4 for fp32,
    ~1.5e-4 max rel err at these contraction depths).
  - x tiles [128, 4096] are DMA'd naturally (rows on partitions), transposed
    128x128 on the TensorEngine (transpose mode), PSUM->SBUF copied by the
    VectorEngine, then used as the stationary operand of f32r matmuls
    against the resident weight tiles.  PSUM accumulates over each block's
    contraction dim; results are copied to an SBUF staging tile and DMA'd
    out in one 2 MiB transfer per 128-row tile.
  - fp32 bits are fed directly into float32r tiles (verified bit-identical
    to explicitly rounded operands on HW - the PE rounds internally).
"""

import numpy as np

import concourse.bacc as bacc
import concourse.mybir as mybir
from concourse.tile import TileContext
from concourse.bass_utils import run_bass_kernel_spmd
from concourse.masks import make_identity

SIZES = [512, 1024, 256, 768, 384, 640, 128, 384]
OFFS = np.cumsum([0] + SIZES)
N_CORES = 8
ROWS_TOTAL = 4 * 2048
ROWS_PER_CORE = ROWS_TOTAL // N_CORES  # 1024
D = 4096
P = 128
R_TILES = ROWS_PER_CORE // P  # 8

# e-slices per block: chunks <=512, all >=256 when possible (f32r matmul
# runs 1 cycle/row only for moving dim >= 256; 512 is the PSUM bank limit)
E_SLICES = {
    512: [512], 1024: [512, 512], 256: [256], 768: [512, 256],
    384: [384], 640: [384, 256], 128: [128],
}

F32R = mybir.dt.float32r
F32 = mybir.dt.float32

_cache = {}


def build_nc():
    if "nc" in _cache:
        return _cache["nc"]
    nc = bacc.Bacc()
    x_d = nc.declare_dram_parameter("x", [ROWS_PER_CORE, D], F32R, isOutput=False)
    w_d = [
        nc.declare_dram_parameter(f"w{i}", [s, s], F32R, isOutput=False)
        for i, s in enumerate(SIZES)
    ]
    out_d = nc.declare_dram_parameter("out", [ROWS_PER_CORE, D], F32, isOutput=True)

    x_v = x_d.rearrange("(r p) d -> r p d", p=P)
    out_v = out_d.rearrange("(r p) d -> r p d", p=P)

    with TileContext(nc) as tc:
        with (
            tc.tile_pool(name="wres", bufs=1) as wres,
            tc.tile_pool(name="xnat", bufs=2) as xnat_p,
            tc.tile_pool(name="xt", bufs=2) as xt_p,
            tc.tile_pool(name="osb", bufs=2) as osb_p,
            tc.tile_pool(name="idp", bufs=1) as idp,
            tc.tile_pool(name="tp", bufs=2, space="PSUM") as tp_p,
            tc.tile_pool(name="mm", bufs=4, space="PSUM") as mm_p,
        ):
            # identity (f32r) for PE transpose
            id32 = idp.tile([P, P], F32, tag="id32")
            make_identity(nc, id32[:])
            ident = idp.tile([P, P], F32R, tag="idr")
            nc.vector.tensor_copy(ident[:], id32[:])

            # resident weights: per block, per k-tile: [128, s] f32r.
            # Issued on the Scalar-engine HWDGE ring so they stream in
            # parallel with x/out DMAs on the Sync ring (the SDMA engines
            # round-robin between the two rings).
            wt = []
            for i, s in enumerate(SIZES):
                w_v = w_d[i].rearrange("(k p) e -> k p e", p=P)
                ks = []
                for k in range(s // P):
                    t = wres.tile([P, s], F32R, tag=f"w{i}_{k}")
                    nc.scalar.dma_start(out=t[:], in_=w_v[k])
                    ks.append(t)
                wt.append(ks)

            for r in range(R_TILES):
                x_t = xnat_p.tile([P, D], F32R, tag="xn")
                nc.sync.dma_start(out=x_t[:, :D // 2], in_=x_v[r][:, :D // 2])
                nc.sync.dma_start(out=x_t[:, D // 2:], in_=x_v[r][:, D // 2:])

                # transpose all 32 d-tiles: xts[j][:, 128*i:...] holds
                # d-tile (4j+i) as [d on partitions, r free]
                xts = []
                for j in range(D // (4 * P)):  # 8 groups of 4
                    ps = tp_p.tile([P, 4 * P], F32R, tag="tp")
                    for i in range(4):
                        nc.tensor.transpose(
                            ps[:, P * i:P * (i + 1)],
                            x_t[:, P * (4 * j + i):P * (4 * j + i + 1)],
                            ident[:],
                        )
                    xt = xt_p.tile([P, 4 * P], F32R, tag=f"xt{j}")
                    nc.vector.tensor_copy(xt[:], ps[:])
                    xts.append(xt)

                o_t = osb_p.tile([P, D], F32, tag="os")
                for b, s in enumerate(SIZES):
                    d0 = int(OFFS[b]) // P  # first global d-tile of block
                    kt = s // P
                    n0 = 0
                    for nw in E_SLICES[s]:
                        ps = mm_p.tile([P, nw], F32, tag="mm")
                        for k in range(kt):
                            g = d0 + k
                            lhsT = xts[g // 4][:, P * (g % 4):P * (g % 4 + 1)]
                            nc.tensor.matmul(
                                ps[:], lhsT, wt[b][k][:, n0:n0 + nw],
                                start=(k == 0), stop=(k == kt - 1),
                            )
                        # alternate PSUM->SBUF output copies between DVE and
                        # ACT so neither engine becomes the bottleneck
                        dst = o_t[:, int(OFFS[b]) + n0:int(OFFS[b]) + n0 + nw]
                        if (r + b) % 2 == 0:
                            nc.scalar.copy(dst, ps[:])
                        else:
                            nc.vector.tensor_copy(dst, ps[:])
                        n0 += nw
                nc.sync.dma_start(out=out_v[r][:, :D // 2], in_=o_t[:, :D // 2])
                nc.sync.dma_start(out=out_v[r][:, D // 2:], in_=o_t[:, D // 2:])

    nc.finalize()
    _cache["nc"] = nc
    return nc


def build_in_maps(x, w0, w1, w2, w3, w4, w5, w6, w7):
    x = np.ascontiguousarray(np.asarray(x, dtype=np.float32)).reshape(ROWS_TOTAL, D)
    ws = [w0, w1, w2, w3, w4, w5, w6, w7]
    wts = [
        np.ascontiguousarray(np.asarray(w, dtype=np.float32).T) for w in ws
    ]
    in_maps = []
    for c in range(N_CORES):
        m = {"x": x[c * ROWS_PER_CORE:(c + 1) * ROWS_PER_CORE]}
        for i, wt in enumerate(wts):
            m[f"w{i}"] = wt
        in_maps.append(m)
    return in_maps


def kernel(x, w0, w1, w2, w3, w4, w5, w6, w7):
    nc = build_nc()
    in_maps = build_in_maps(x, w0, w1, w2, w3, w4, w5, w6, w7)
    res = run_bass_kernel_spmd(nc, in_maps, list(range(N_CORES)))
    out = np.concatenate([r["out"] for r in res.results], axis=0)
    return out.reshape(4, 2048, D).astype(np.float32, copy=False)
